# revision 12
# baseline (speedup 1.0000x reference)
import sys

if "/opt/trn_rl_repo" not in sys.path:
    sys.path.insert(0, "/opt/trn_rl_repo")

import numpy as np
import ml_dtypes

import concourse.bass as bass
import concourse.bacc as bacc
import concourse.tile as tile
import concourse.mybir as mybir
from concourse import bass_utils

# Problem shapes (nn_ChebConv): x (16, 12288), L (12288, 12288),
# weights (5, 16, 32), bias (32,). out (32, 12288).
#
# Sharding: core d owns V-columns [d*1536, (d+1)*1536).  Host feeds each
# core ltA = L^T[:, cols_d[0:1024]] and ltB = L^T[:, cols_d[1024:1536]]
# (contraction dim on partitions), row-interleaved within 512-row groups
# so each SBUF partition reads one contiguous chunk per bulk DMA.
#
# Two-phase schedule per Chebyshev step: phase A contracts all 96
# vc-tiles into psum chunks 0,1 (output cols 0:1024); phase B into
# chunk 2 (cols 1024:1536).  Chunks 0,1 therefore close at ~2/3 of the
# step, so the AllGather of T_k^T[0:1024] (AG-A) runs UNDER phase B and
# lands before the step ends; AG-B (cols 1024:1536) trails the step by
# a few us but its data (contraction groups g%3==2) is ordered ~27us
# into the next step.  Boundaries between steps cost ~0: the next
# step's first matmul issues immediately after the last.
C_IN = 16
C_OUT = 32
K_CHEB = 5
V = 12288
N_CORES = 8
VLOC = V // N_CORES          # 1536 columns of the V axis per core
P = 128
NB = 4                       # vc-tiles per lt group (512 rows)
NG = V // (P * NB)           # 24 contraction groups
CA = 1024                    # phase-A output cols (psum chunks 0,1)
CB = 512                     # phase-B output cols (chunk 2)
N_SG = 11                    # streamed groups (0..10); 11..23 resident
LTA_BUFS = 3
LTB_BUFS = 2

_CACHE: dict = {}


def _orders():
    S = list(range(N_SG))
    R = list(range(N_SG, NG))

    def ilv(a, b):
        out = []
        for x, y in zip(a, b):
            out += [x, y]
        n = min(len(a), len(b))
        return out + a[n:] + b[n:]

    # resident-first interleave; groups with g%3==2 (whose stationary
    # tiles come from AG-B) go last so AG-B has ~27us of slack
    a_non = ilv([g for g in R if g % 3 != 2], [g for g in S if g % 3 != 2])
    a_c2 = ilv([g for g in S if g % 3 == 2], [g for g in R if g % 3 == 2])
    order = a_non + a_c2
    # last slot whose group is streamed: anchor for the statA gate
    gate_slot = max(i for i, g in enumerate(order) if g < N_SG)
    return order, gate_slot


def _build(cfg: str):
    if cfg == "bf16":
        mm_dt = mybir.dt.bfloat16
    else:
        mm_dt = mybir.dt.float32
    f32 = mybir.dt.float32

    order_g, gate_slot = _orders()

    nc = bacc.Bacc("TRN2", target_bir_lowering=False, debug=False,
                   num_devices=N_CORES)

    ltA = nc.dram_tensor("ltA", [V, CA], mm_dt, kind="ExternalInput")
    ltB = nc.dram_tensor("ltB", [V, CB], mm_dt, kind="ExternalInput")
    xtA = nc.dram_tensor("xtA", [N_CORES * CA, C_IN], mm_dt,
                         kind="ExternalInput")
    xtB = nc.dram_tensor("xtB", [N_CORES * CB, C_IN], mm_dt,
                         kind="ExternalInput")
    xc = nc.dram_tensor("xc", [C_IN, VLOC], mm_dt, kind="ExternalInput")
    wf = nc.dram_tensor("wf", [P, C_OUT], mm_dt, kind="ExternalInput")
    w4 = nc.dram_tensor("w4", [C_IN, C_OUT], mm_dt, kind="ExternalInput")
    bias_in = nc.dram_tensor("bias_in", [1, C_OUT], f32, kind="ExternalInput")
    id128 = nc.dram_tensor("id128", [P, C_IN], mm_dt, kind="ExternalInput")
    out = nc.dram_tensor("out", [C_OUT, VLOC], f32, kind="ExternalOutput")

    ltA_r = ltA.ap().rearrange("(g p u) c -> g p u c", p=P, u=NB)
    ltB_r = ltB.ap().rearrange("(g p u) c -> g p u c", p=P, u=NB)
    xtA_r = xtA.ap().rearrange("(g p j) c -> p g j c", p=P, j=8)
    xtB_r = xtB.ap().rearrange("(g p j) c -> p g j c", p=P, j=4)

    with tile.TileContext(nc) as tc:
        with (
            tc.tile_pool(name="ltpA", bufs=LTA_BUFS) as ltpA,
            tc.tile_pool(name="ltpB", bufs=LTB_BUFS) as ltpB,
            tc.tile_pool(name="persist", bufs=1) as persist,
            tc.tile_pool(name="stat", bufs=2) as statp,
            tc.tile_pool(name="work", bufs=2) as work,
            tc.tile_pool(name="acc", bufs=4, space="PSUM") as accp,
            tc.tile_pool(name="tpp", bufs=4, space="PSUM") as tpp,
            tc.tile_pool(name="dram", bufs=1, space="DRAM") as dram,
        ):
            # ---- persistent small tensors (scalar ring) ----
            w_sb = persist.tile([P, C_OUT], mm_dt)
            nc.scalar.dma_start(w_sb[:], wf.ap())
            w4_sb = persist.tile([C_IN, C_OUT], mm_dt)
            nc.scalar.dma_start(w4_sb[:], w4.ap())
            bias_sb = persist.tile([1, C_OUT], f32)
            nc.scalar.dma_start(bias_sb[:], bias_in.ap())
            ones_sb = persist.tile([1, 512], f32)
            nc.vector.memset(ones_sb[:], 1.0)
            id_sb = persist.tile([P, C_IN], mm_dt)
            nc.scalar.dma_start(id_sb[:], id128.ap())

            # tiny warm-up AllGather: triggers the one-time CC-ring setup
            # (~55us) immediately; all-gpsimd so it fires at ~9us with no
            # cross-engine deps (low trigger skew across ranks)
            wu_sb = persist.tile([P, C_IN], mm_dt)
            nc.gpsimd.memset(wu_sb[:], 0.0)
            wu_in = dram.tile([P, C_IN], mm_dt, name="wu_in0")
            wu_out = dram.tile([P * N_CORES, C_IN], mm_dt, name="wu_out0",
                               addr_space="Shared")
            nc.gpsimd.dma_start(wu_in[:], wu_sb[:])
            nc.gpsimd.collective_compute(
                "AllGather",
                mybir.AluOpType.bypass,
                replica_groups=[list(range(N_CORES))],
                ins=[wu_in.opt()],
                outs=[wu_out.opt()],
            )

            # T_0..T_3 stacked at partition bases {0,32,64,96} of one tile
            # (rows 16-31 of each block stay zero; the fused einsum
            # contracts all 128 partitions against zero-padded weights).
            t_blk = persist.tile([P, VLOC], mm_dt)
            t4_sb = persist.tile([C_IN, VLOC], mm_dt)
            nc.vector.memset(t_blk[:], 0.0)
            nc.scalar.dma_start(t_blk[0:C_IN, :], xc.ap())

            def t_ap(k):
                if k == K_CHEB - 1:
                    return t4_sb[:]
                return t_blk[32 * k:32 * k + C_IN, :]

            # stationary tiles: statA holds T^T rows for within-rank
            # vertex blocks 0..7 of all 8 ranks (rank-major), statB blocks
            # 8..11.  ONE DMA each: long-pending stat loads (gated on the
            # AllGather) must not hold more than one DMA semaphore slot,
            # or the free-flowing lt stream stalls on slot recycling.
            def load_stat(src_r, tag_k, part):
                nj = 8 if part == "A" else 4
                s = statp.tile([P, N_CORES * nj * C_IN], mm_dt,
                               name=f"s{part}{tag_k}", tag=f"stat{part}")
                nc.scalar.dma_start(
                    s[:].rearrange("p (g j c) -> p g j c", g=N_CORES, j=nj),
                    src_r)
                return s

            curA = load_stat(xtA_r, 0, "A")
            curB = load_stat(xtB_r, 0, "B")

            # resident lt pieces for groups 12..23 (loaded JIT during
            # step 1 on the sync ring, reused DMA-free by steps 2-4)
            resA = [persist.tile([P, NB * CA], mm_dt, name=f"rsA{i}")
                    for i in range(NG - N_SG)]
            resB = [persist.tile([P, NB * CB], mm_dt, name=f"rsB{i}")
                    for i in range(NG - N_SG)]

            def get_ltA(k, g):
                if g >= N_SG:
                    src = resA[g - N_SG]
                    if k == 1:
                        nc.sync.dma_start(
                            src[:].rearrange("p (u c) -> p u c", u=NB),
                            ltA_r[g])
                    return src
                src = ltpA.tile([P, NB * CA], mm_dt,
                                name=f"lA{k}_{g}", tag="lA")
                nc.sync.dma_start(
                    src[:].rearrange("p (u c) -> p u c", u=NB), ltA_r[g])
                return src

            def get_ltB(k, g):
                if g >= N_SG:
                    src = resB[g - N_SG]
                    if k == 1:
                        nc.sync.dma_start(
                            src[:].rearrange("p (u c) -> p u c", u=NB),
                            ltB_r[g])
                    return src
                src = ltpB.tile([P, NB * CB], mm_dt,
                                name=f"lB{k}_{g}", tag="lB")
                nc.sync.dma_start(
                    src[:].rearrange("p (u c) -> p u c", u=NB), ltB_r[g])
                return src

            def stat_ap(g, u):
                j = g * NB + u
                r, us = j // 12, j % 12
                if us < 8:
                    m = r * 8 + us
                    return curA[:, m * C_IN:(m + 1) * C_IN]
                m = r * 4 + (us - 8)
                return curB[:, m * C_IN:(m + 1) * C_IN]

            pending_B = None

            def emit_load_statB():
                coB_r, kk = pending_B
                return load_stat(coB_r, kk, "B")

            for k in range(1, K_CHEB):
                accA = [accp.tile([C_IN, 512], f32, name=f"accA{k}_{ch}",
                                  tag="acc") for ch in range(2)]
                accB = accp.tile([C_IN, 512], f32, name=f"accB{k}",
                                 tag="acc")
                tb = 32 * k
                last = K_CHEB - 1

                # ---------- phase A: psum chunks 0,1 over all groups ----
                for gi, g in enumerate(order_g):
                    src = get_ltA(k, g)
                    if gi == 3 and pending_B is not None:
                        # gate: hold the scalar queue on this streamed
                        # tile (~+5us) so load_statB never pends long on
                        # AG-B -- long-pending DMAs poison the rotating
                        # DMA-semaphore pool and stall unrelated streams
                        gt = work.tile([1, 64], mm_dt, name=f"gB{k}",
                                       tag="gate")
                        nc.scalar.dma_start(gt[:], src[0:1, 0:64])
                        curB = emit_load_statB()
                        pending_B = None
                    for u in range(NB):
                        for ch in range(2):
                            nc.tensor.matmul(
                                accA[ch][:],
                                lhsT=stat_ap(g, u),
                                rhs=src[:, u * CA + ch * 512:
                                        u * CA + (ch + 1) * 512],
                                start=(gi == 0 and u == 0),
                                stop=(gi == NG - 1 and u == NB - 1))

                # DVE: T_k chunks 0,1 (runs under phase-B group 0)
                def emit_dve(ch, acc):
                    sl = slice(ch * 512, (ch + 1) * 512)
                    if k == 1:
                        nc.vector.tensor_copy(t_ap(k)[:, sl], acc[:])
                    else:
                        nc.vector.scalar_tensor_tensor(
                            t_ap(k)[:, sl], acc[:], 2.0,
                            t_ap(k - 2)[:, sl],
                            mybir.AluOpType.mult, mybir.AluOpType.subtract)

                if k < last:
                    scA = work.tile([P, 8 * C_IN], mm_dt,
                                    name=f"scA{k}", tag="scs")
                    scB = work.tile([P, 4 * C_IN], mm_dt,
                                    name=f"scB{k}", tag="scs")
                    cc_inA = dram.tile([CA, C_IN], mm_dt, name=f"ccA{k}")
                    cc_inB = dram.tile([CB, C_IN], mm_dt, name=f"ccB{k}")

                def emit_tp(j2, sc, col):
                    tp_ps = tpp.tile([P, C_IN], mm_dt,
                                     name=f"tp{k}_{j2}", tag="tp")
                    nc.tensor.transpose(
                        tp_ps[:],
                        t_blk[tb:tb + C_IN, j2 * P:(j2 + 1) * P],
                        id_sb[tb:tb + C_IN, :],
                        tile_position=(tb, 0) if tb == 96 else None)
                    nc.vector.tensor_copy(
                        sc[:, col * C_IN:(col + 1) * C_IN], tp_ps[:])

                def emit_ein(ch):
                    # fused einsum for psum chunk ch: bias + T_0..T_3
                    # (one 128-part matmul) + T_4 term, then copy out
                    sl = slice(ch * 512, (ch + 1) * 512)
                    ein = tpp.tile([C_OUT, 512], f32, name=f"ein{ch}",
                                   tag="tp")
                    nc.tensor.matmul(ein[:], lhsT=bias_sb[:],
                                     rhs=ones_sb[:], start=True, stop=False)
                    nc.tensor.matmul(ein[:], lhsT=w_sb[:],
                                     rhs=t_blk[:, sl],
                                     start=False, stop=False)
                    nc.tensor.matmul(ein[:], lhsT=w4_sb[:],
                                     rhs=t4_sb[:, sl],
                                     start=False, stop=True)
                    res = work.tile([C_OUT, 512], f32, name=f"res{ch}",
                                    tag="res")
                    nc.vector.tensor_copy(res[:], ein[:])
                    nc.scalar.dma_start(
                        out.ap()[:, ch * 512:(ch + 1) * 512], res[:])

                # ---------- phase B: psum chunk 2; AG-A chain under it --
                nextA = None
                for gi, g in enumerate(order_g):
                    src = get_ltB(k, g)
                    for u in range(NB):
                        nc.tensor.matmul(
                            accB[:],
                            lhsT=stat_ap(g, u),
                            rhs=src[:, u * CB:(u + 1) * CB],
                            start=(gi == 0 and u == 0),
                            stop=(gi == NG - 1 and u == NB - 1))
                    if gi == 0:
                        emit_dve(0, accA[0])
                        emit_dve(1, accA[1])
                    elif gi == 1:
                        if k == last:
                            emit_ein(0)
                        else:
                            for j2 in range(4):
                                emit_tp(j2, scA, j2)
                    elif gi == 2:
                        if k == last:
                            emit_ein(1)
                        else:
                            for j2 in range(4, 8):
                                emit_tp(j2, scA, j2)
                            nc.scalar.dma_start(
                                cc_inA.rearrange("(p j) c -> p j c", p=P),
                                scA[:].rearrange("p (j c) -> p j c", j=8))
                            cc_outA = dram.tile([N_CORES * CA, C_IN], mm_dt,
                                                name=f"ccoA{k}",
                                                addr_space="Shared")
                            nc.gpsimd.collective_compute(
                                "AllGather",
                                mybir.AluOpType.bypass,
                                replica_groups=[list(range(N_CORES))],
                                ins=[cc_inA.opt()],
                                outs=[cc_outA.opt()],
                            )
                            coA_r = cc_outA.rearrange(
                                "(g p j) c -> p g j c", p=P, j=8)
                    elif gi == gate_slot and k < last:
                        # gate on this streamed tile (~4us before step
                        # end) so load_statA issues when AG-A is done
                        gt = work.tile([1, 64], mm_dt, name=f"gA{k}",
                                       tag="gate")
                        nc.scalar.dma_start(gt[:], src[0:1, 0:64])
                        nextA = load_stat(coA_r, k, "A")

                # tail: T_k chunk 2, then AG-B (or final einsum chunk 2)
                emit_dve(2, accB)
                if k == last:
                    emit_ein(2)
                else:
                    for j2 in range(8, 12):
                        emit_tp(j2, scB, j2 - 8)
                    nc.scalar.dma_start(
                        cc_inB.rearrange("(p j) c -> p j c", p=P),
                        scB[:].rearrange("p (j c) -> p j c", j=4))
                    cc_outB = dram.tile([N_CORES * CB, C_IN], mm_dt,
                                        name=f"ccoB{k}",
                                        addr_space="Shared")
                    nc.gpsimd.collective_compute(
                        "AllGather",
                        mybir.AluOpType.bypass,
                        replica_groups=[list(range(N_CORES))],
                        ins=[cc_inB.opt()],
                        outs=[cc_outB.opt()],
                    )
                    coB_r = cc_outB.rearrange("(g p j) c -> p g j c",
                                              p=P, j=4)
                    pending_B = (coB_r, k)
                    curA = nextA

    nc.compile()
    return nc


def _interleave_rows(a, nb):
    """Within each nb*128-row group, reorder rows so row g*G+nb*p+u holds
    original row g*G+u*128+p (one contiguous per-partition read)."""
    ng = a.shape[0] // (P * nb)
    return np.ascontiguousarray(
        a.reshape(ng, nb, P, a.shape[1]).transpose(0, 2, 1, 3)
        .reshape(a.shape))


def _stat_interleave(xt, span):
    """xt: (V, C) -> per-rank blocks of `span` rows taken from the head
    (span=1024) or tail (span=512) of each rank's 1536 rows, interleaved
    in j-groups of 128 so partition p reads span//128 consecutive rows."""
    nj = span // P
    blocks = []
    for r in range(N_CORES):
        if span == CA:
            blk = xt[r * VLOC: r * VLOC + CA]
        else:
            blk = xt[r * VLOC + CA: (r + 1) * VLOC]
        blocks.append(blk.reshape(nj, P, xt.shape[1])
                      .transpose(1, 0, 2).reshape(span, xt.shape[1]))
    return np.ascontiguousarray(np.concatenate(blocks, axis=0))


def _prep_inputs(x, L, weights, bias, cfg: str):
    np_dt = ml_dtypes.bfloat16 if cfg == "bf16" else np.float32
    x = np.asarray(x, dtype=np.float32)
    L = np.asarray(L, dtype=np.float32)
    weights = np.asarray(weights, dtype=np.float32)
    bias = np.asarray(bias, dtype=np.float32)

    Lt = np.ascontiguousarray(L.T).astype(np_dt)          # (V, V)
    xt = np.ascontiguousarray(x.T).astype(np_dt)          # (V, C_IN)
    xtA_h = _stat_interleave(xt, CA)
    xtB_h = _stat_interleave(xt, CB)

    wf = np.zeros((P, C_OUT), dtype=np_dt)
    for k in range(K_CHEB - 1):
        wf[32 * k:32 * k + C_IN, :] = weights[k]
    w4 = np.ascontiguousarray(weights[K_CHEB - 1]).astype(np_dt)
    b_ = np.ascontiguousarray(bias.reshape(1, C_OUT))
    id128 = np.zeros((P, C_IN), dtype=np_dt)
    for p in range(P):
        if p % 32 < C_IN:
            id128[p, p % 32] = 1.0

    in_maps = []
    for d in range(N_CORES):
        c0 = d * VLOC
        in_maps.append({
            "ltA": _interleave_rows(
                np.ascontiguousarray(Lt[:, c0:c0 + CA]), NB),
            "ltB": _interleave_rows(
                np.ascontiguousarray(Lt[:, c0 + CA:c0 + VLOC]), NB),
            "xtA": xtA_h,
            "xtB": xtB_h,
            "xc": np.ascontiguousarray(x[:, c0:c0 + VLOC]).astype(np_dt),
            "wf": wf,
            "w4": w4,
            "bias_in": b_,
            "id128": id128,
        })
    return in_maps


def run(x, L, weights, bias, cfg: str = "bf16", trace: bool = False,
        trace_cores=None):
    if cfg not in _CACHE:
        _CACHE[cfg] = _build(cfg)
    nc = _CACHE[cfg]
    in_maps = _prep_inputs(x, L, weights, bias, cfg)
    kw = {}
    if trace_cores is not None:
        kw["trace_cores"] = trace_cores
    res = bass_utils.run_bass_kernel_spmd(
        nc, in_maps, core_ids=list(range(N_CORES)), trace=trace, **kw)
    out = np.concatenate([res.results[d]["out"] for d in range(N_CORES)],
                         axis=1)
    return out.astype(np.float32), res


def kernel(x, L, weights, bias):
    out, _ = run(x, L, weights, bias, cfg="bf16")
    return out
